# revision 32
# baseline (speedup 1.0000x reference)
"""Trainium2 Bass kernel for EnhancedConditionalUNet forward (B=64, 8 cores data-parallel).

Self-contained: hardcodes all shapes. kernel(**inputs) -> np.ndarray [64,3,64,64] f32.

Design: per-sample software pipeline on each core (8 samples/core).
- conv chain (e1 im2col, e2 stride-2 via even/odd x-planes, b1, b2, deconv-as-4-phase,
  dc col-packed across 4 PE column groups) in fp16 matmuls with fp32 PSUM accumulation
- attention (q,k fp16; bf16 attention weights) computed in transposed layout (j on
  partitions) to avoid transposes; softmax without max-subtraction (scores are small);
  colsum via ones-matmul; row-broadcast of 1/colsum via PE replication
- two-stage pipeline: sample s's front half (e1..attention) is emitted between sample
  s-1's normalization tail and back half (b2..dc), hiding the reciprocal serial chain
"""
import numpy as np

import concourse.bass as bass
import concourse.tile as tile
from concourse import bacc, mybir
from concourse.bass_utils import run_bass_kernel_spmd

NCORES = 8
NS = 8          # samples per core
BF = mybir.dt.bfloat16
F16 = mybir.dt.float16
F32 = mybir.dt.float32
AF = mybir.ActivationFunctionType
OP = mybir.AluOpType

_cache = {}


def build_nc(ns=NS):
    nc = bacc.Bacc("TRN2", target_bir_lowering=False, debug=False)

    d_m0 = nc.dram_tensor("m0", [ns, 36, 64, 64], F16, kind="ExternalInput")
    d_wim = nc.dram_tensor("wim", [36, 64], F16, kind="ExternalInput")
    d_be1 = nc.dram_tensor("be1", [64, 1], F32, kind="ExternalInput")
    d_we2 = nc.dram_tensor("we2", [64, 9, 128], F16, kind="ExternalInput")
    d_be2 = nc.dram_tensor("be2", [128, 1], F32, kind="ExternalInput")
    d_wb1 = nc.dram_tensor("wb1", [128, 9, 2, 128], F16, kind="ExternalInput")
    d_bb1 = nc.dram_tensor("bb1", [128, 2], F32, kind="ExternalInput")
    d_wq = nc.dram_tensor("wq", [128, 2, 32], F16, kind="ExternalInput")
    d_bq = nc.dram_tensor("bq", [32, 1], F32, kind="ExternalInput")
    d_wk = nc.dram_tensor("wk", [128, 2, 32], F16, kind="ExternalInput")
    d_bk = nc.dram_tensor("bk", [32, 1], F32, kind="ExternalInput")
    d_wv = nc.dram_tensor("wv", [128, 2, 256], F16, kind="ExternalInput")
    d_gvb = nc.dram_tensor("gvb", [128, 2], F32, kind="ExternalInput")
    d_gam = nc.dram_tensor("gam", [1, 1], F32, kind="ExternalInput")
    d_wb2 = nc.dram_tensor("wb2", [128, 2, 9, 128], F16, kind="ExternalInput")
    d_bb2 = nc.dram_tensor("bb2", [128, 1], F32, kind="ExternalInput")
    d_wdt = nc.dram_tensor("wdt", [128, 16, 64], F16, kind="ExternalInput")
    d_bdt = nc.dram_tensor("bdt", [64, 1], F32, kind="ExternalInput")
    d_wdc = nc.dram_tensor("wdc", [64, 9, 3], F16, kind="ExternalInput")
    d_bdc = nc.dram_tensor("bdc", [128, 1], F32, kind="ExternalInput")
    d_out = nc.dram_tensor("out", [ns, 3, 64, 64], F32, kind="ExternalOutput")

    with tile.TileContext(nc) as tc:
        with (
            tc.tile_pool(name="wpool", bufs=1) as wp,
            tc.tile_pool(name="apool", bufs=2) as ap,
            tc.tile_pool(name="spool", bufs=1) as sp,
            tc.tile_pool(name="psS", bufs=4, space="PSUM") as psS,
            tc.tile_pool(name="psB", bufs=2, space="PSUM") as psB,
        ):
            _eng = [nc.gpsimd, nc.scalar]
            _ei = [0]

            def wload(name, shape, dt, dram, split=1):
                t = wp.tile(shape, dt, name=name)
                n0 = shape[0]
                step = (n0 + split - 1) // split
                for o in range(0, n0, step):
                    e = _eng[_ei[0] % len(_eng)]
                    _ei[0] += 1
                    e.dma_start(t[o:o + step], dram[o:o + step])
                return t

            wim = wload("wim", [36, 64], F16, d_wim)
            be1 = wload("be1", [64, 1], F32, d_be1)
            we2 = wload("we2", [64, 9, 128], F16, d_we2)
            be2 = wload("be2", [128, 1], F32, d_be2)
            wb1 = wload("wb1", [128, 9, 2, 128], F16, d_wb1, split=4)
            bb1 = wload("bb1", [128, 2], F32, d_bb1)
            wq = wload("wq", [128, 2, 32], F16, d_wq)
            bq = wload("bq", [32, 1], F32, d_bq)
            wk = wload("wk", [128, 2, 32], F16, d_wk)
            bk = wload("bk", [32, 1], F32, d_bk)
            wv = wload("wv", [128, 2, 256], F16, d_wv, split=2)
            gvb = wload("gvb", [128, 2], F32, d_gvb)
            gam = wload("gam", [1, 1], F32, d_gam)
            ones128 = wp.tile([128, 1], BF)
            nc.vector.memset(ones128[:], 1.0)
            ones1 = wp.tile([1, 128], BF)
            nc.vector.memset(ones1[:], 1.0)

            taps = [(dy, dx) for dy in range(3) for dx in range(3)]

            def borders(t, H, W):
                nc.gpsimd.memset(t[:, 0, :], 0.0)
                nc.gpsimd.memset(t[:, H - 1, :], 0.0)
                nc.gpsimd.memset(t[:, 1:H - 1, 0], 0.0)
                nc.gpsimd.memset(t[:, 1:H - 1, W - 1], 0.0)

            def stage_a(s):
                """e1..attention front half. Returns state for the tail/back half."""
                # e1: im2col [36,4096] -> relu -> h1p even/odd planes [64,66,2,33]
                m0 = ap.tile([36, 64, 64], F16, name="m0t", bufs=3)
                _m0eng = [nc.sync, nc.gpsimd, nc.scalar, nc.sync]
                for t4 in range(4):
                    _m0eng[t4].dma_start(m0[9 * t4:9 * t4 + 9, :, :],
                                         d_m0[s, 9 * t4:9 * t4 + 9])
                h1p = ap.tile([64, 66, 2, 33], F16, name="h1p")
                nc.gpsimd.memset(h1p[:, 0, :, :], 0.0)
                nc.gpsimd.memset(h1p[:, 65, :, :], 0.0)
                nc.gpsimd.memset(h1p[:, 1:65, 0, 0], 0.0)
                nc.gpsimd.memset(h1p[:, 1:65, 1, 32], 0.0)
                for r in range(8):
                    ps = psS.tile([64, 512], F32, name="pcs")
                    nc.tensor.matmul(ps[:], wim[:], m0[:, 8 * r:8 * r + 8, :],
                                     start=True, stop=True)
                    # m0 cols pre-permuted on host: per row, first 32 -> plane0 xx1..32,
                    # last 32 -> plane1 xx0..31; h1p flat row addr (p*33+xx) = 1..64 contiguous
                    h1f = h1p.rearrange("p a b c -> p a (b c)")
                    nc.scalar.activation(h1f[:, 1 + 8 * r:9 + 8 * r, 1:65],
                                         ps[:].rearrange("p (a b) -> p a b", a=8),
                                         AF.Relu, bias=be1[:], scale=1.0)

                # e2: stride2 64->32 via planes, K=64 -> h2 [128,34,34]
                h2 = ap.tile([128, 34, 34], F16, name="h2")
                borders(h2, 34, 34)
                for r in range(2):
                    ps = psS.tile([128, 512], F32, name="pcs")
                    for ti, (dy, dx) in enumerate(taps):
                        rhs = h1p[:, dy + 32 * r:dy + 32 * r + 32:2,
                                  dx % 2, dx // 2:dx // 2 + 32]
                        nc.tensor.matmul(ps[:], we2[:, ti, :], rhs,
                                         start=(ti == 0), stop=(ti == 8))
                    nc.vector.tensor_scalar(out=h2[:, 1 + 16 * r:17 + 16 * r, 1:33],
                                            in0=ps[:], scalar1=be2[:], scalar2=0.0,
                                            op0=OP.add, op1=OP.max)

                # b1: K=128, M=256 -> h3 [128,2,32,32]
                h3 = ap.tile([128, 2, 32, 32], F16, name="h3")
                for mh in range(2):
                    for r in range(2):
                        ps = psS.tile([128, 512], F32, name="pcs")
                        for ti, (dy, dx) in enumerate(taps):
                            nc.tensor.matmul(
                                ps[:], wb1[:, ti, mh, :],
                                h2[:, dy + 16 * r:dy + 16 * r + 16, dx:dx + 32],
                                start=(ti == 0), stop=(ti == 8))
                        nc.vector.tensor_scalar(
                            out=h3[:, mh, 16 * r:16 * r + 16, :].rearrange("p a b -> p (a b)"),
                            in0=ps[:], scalar1=bb1[:, mh:mh + 1], scalar2=0.0,
                            op0=OP.add, op1=OP.max)
                h3f = h3.rearrange("p m a b -> p m (a b)")

                # q, k
                qsb = ap.tile([32, 1024], F16, name="qsb")
                ksb = ap.tile([32, 1024], F16, name="ksb")
                for (wt, bt, dst) in ((wq, bq, qsb), (wk, bk, ksb)):
                    for r in range(2):
                        ps = psS.tile([32, 512], F32, name="pcs")
                        for kh in range(2):
                            nc.tensor.matmul(ps[:], wt[:, kh, :],
                                             h3f[:, kh, 512 * r:512 * r + 512],
                                             start=(kh == 0), stop=(kh == 1))
                        nc.vector.tensor_scalar(out=dst[:, 512 * r:512 * r + 512],
                                                in0=ps[:], scalar1=bt[:], scalar2=None,
                                                op0=OP.add)

                # vT [128,8,256] bf16
                vT = ap.tile([128, 8, 256], BF, name="vT")
                for cc in range(8):
                    ps = psS.tile([128, 256], F32, name="pcs")
                    for kh in range(2):
                        nc.tensor.matmul(ps[:], h3f[:, kh, 128 * cc:128 * cc + 128],
                                         wv[:, kh, :], start=(kh == 0), stop=(kh == 1))
                    nc.vector.tensor_copy(vT[:, cc, :], ps[:])

                # S_T + exp -> E
                E = ap.tile([128, 8, 1024], BF, name="E", bufs=1)
                for cc in range(8):
                    sps = psB.tile([128, 1024], F32, name="pbig")
                    for ih in range(2):
                        nc.tensor.matmul(sps[:, 512 * ih:512 * ih + 512],
                                         ksb[:, 128 * cc:128 * cc + 128],
                                         qsb[:, 512 * ih:512 * ih + 512],
                                         start=True, stop=True)
                    nc.scalar.activation(E[:, cc, :], sps[:], AF.Exp)

                return dict(s=s, h3f=h3f, E=E, vT=vT)

            def stage_a2(st):
                h3f, E, vT = st["h3f"], st["E"], st["vT"]
                # colsum + 1/x * gamma (DVE chain overlaps next emissions)
                cs = psB.tile([1, 1024], F32, name="pbig")
                for cc in range(8):
                    for ih in range(2):
                        nc.tensor.matmul(cs[:, 512 * ih:512 * ih + 512],
                                         ones128[:], E[:, cc, 512 * ih:512 * ih + 512],
                                         start=(cc == 0), stop=(cc == 7))
                csb = sp.tile([1, 1024], F32, name="csb")
                nc.vector.tensor_copy(csb[:], cs[:])
                inv = sp.tile([1, 1024], F32, name="inv")
                scr = sp.tile([1, 1024], F32, name="scr")
                nc.vector.reciprocal_approx_accurate(inv[:], csb[:], scr[:])
                invg = ap.tile([1, 1024], BF, name="invg")
                nc.vector.tensor_scalar(out=invg[:], in0=inv[:], scalar1=gam[:],
                                        scalar2=None, op0=OP.mult)
                # attn out (v.E) per c-half -> atsb (f32, SBUF)
                atsb = ap.tile([128, 2, 1024], F32, name="atsb")
                for ch in range(2):
                    at = psB.tile([128, 1024], F32, name="pbig")
                    for ih in range(2):
                        for cc in range(8):
                            nc.tensor.matmul(at[:, 512 * ih:512 * ih + 512],
                                             vT[:, cc, 128 * ch:128 * ch + 128],
                                             E[:, cc, 512 * ih:512 * ih + 512],
                                             start=(cc == 0), stop=(cc == 7))
                    nc.vector.tensor_copy(atsb[:, ch, :], at[:])

                st["atsb"] = atsb
                st["invg"] = invg

            def emit_rep(st):
                invg = st["invg"]
                rep = psB.tile([128, 1024], F32, name="pbig")
                for ih in range(2):
                    nc.tensor.matmul(rep[:, 512 * ih:512 * ih + 512], ones1[:],
                                     invg[:, 512 * ih:512 * ih + 512],
                                     start=True, stop=True)
                repsb = sp.tile([128, 1024], F32, name="repsb")
                nc.vector.tensor_copy(repsb[:], rep[:])
                st["repsb"] = repsb

            def stage_norm(st):
                """Apply 1/colsum*gamma + residual -> hb2 (rep precomputed or here)."""
                if "repsb" not in st:
                    emit_rep(st)
                invg, atsb, h3f = st["invg"], st["atsb"], st["h3f"]
                repsb = st["repsb"]
                hb2 = ap.tile([128, 2, 34, 34], F16, name="hb2")
                for ch in range(2):
                    borders(hb2[:, ch], 34, 34)
                    t1 = sp.tile([128, 1024], F32, name="t1")
                    nc.vector.tensor_mul(t1[:], atsb[:, ch, :], repsb[:])
                    nc.vector.scalar_tensor_tensor(
                        out=hb2[:, ch, 1:33, 1:33],
                        in0=t1[:], scalar=gvb[:, ch:ch + 1], in1=h3f[:, ch, :],
                        op0=OP.add, op1=OP.add)
                st["hb2"] = hb2

            def stage_b(st):
                """b2, deconv, dc, tanh, store."""
                wb2, bb2 = late["wb2"], late["bb2"]
                wdt, bdt = late["wdt"], late["bdt"]
                wdc, bdc = late["wdc"], late["bdc"]
                s, hb2 = st["s"], st["hb2"]
                h4 = ap.tile([128, 34, 34], F16, name="h4")
                borders(h4, 34, 34)
                for r in range(2):
                    ps = psS.tile([128, 512], F32, name="pcs")
                    for kh in range(2):
                        for ti, (dy, dx) in enumerate(taps):
                            nc.tensor.matmul(
                                ps[:], wb2[:, kh, ti, :],
                                hb2[:, kh, dy + 16 * r:dy + 16 * r + 16, dx:dx + 32],
                                start=(kh == 0 and ti == 0), stop=(kh == 1 and ti == 8))
                    nc.vector.tensor_scalar(out=h4[:, 1 + 16 * r:17 + 16 * r, 1:33],
                                            in0=ps[:], scalar1=bb2[:], scalar2=0.0,
                                            op0=OP.add, op1=OP.max)

                h5 = ap.tile([64, 66, 66], F16, name="h5")
                borders(h5, 66, 66)
                aoff = {(0, 0): 1, (0, 1): 0, (1, 0): 2, (1, 1): 1}
                for py in range(2):
                    for px in range(2):
                        ph = py * 2 + px
                        for r in range(2):
                            ps = psS.tile([64, 512], F32, name="pcs")
                            ti = 0
                            for dy2 in range(2):
                                for dx2 in range(2):
                                    ay = aoff[(py, dy2)]
                                    ax = aoff[(px, dx2)]
                                    nc.tensor.matmul(
                                        ps[:], wdt[:, ph * 4 + dy2 * 2 + dx2, :],
                                        h4[:, ay + 16 * r:ay + 16 * r + 16, ax:ax + 32],
                                        start=(ti == 0), stop=(ti == 3))
                                    ti += 1
                            out_ap = h5[:, 1 + py + 32 * r:1 + py + 32 * r + 32:2,
                                        1 + px:1 + px + 64:2]
                            nc.scalar.activation(out_ap, ps[:], AF.Relu,
                                                 bias=bdt[:], scale=1.0)

                dct = ap.tile([128, 1024], F32, name="dct")
                for r2 in range(2):
                    qt = [psS.tile([128, 512], F32, name="pcs") for _ in range(4)]
                    for ti, (dy, dx) in enumerate(taps):
                        for Q in range(4):
                            y0 = 16 * Q + 8 * r2
                            nc.tensor.matmul(
                                qt[Q][32 * Q:32 * Q + 3, :],
                                wdc[:, ti, :],
                                h5[:, dy + y0:dy + y0 + 8, dx:dx + 64],
                                start=(ti == 0), stop=(ti == 8),
                                tile_position=(0, 32 * Q))
                    for Q in range(4):
                        nc.scalar.activation(dct[32 * Q:32 * Q + 3, 512 * r2:512 * r2 + 512],
                                             qt[Q][32 * Q:32 * Q + 3, :], AF.Tanh,
                                             bias=bdc[32 * Q:32 * Q + 3, :], scale=1.0)
                for Q in range(4):
                    nc.sync.dma_start(d_out[s][:, 16 * Q:16 * Q + 16, :],
                                      dct[32 * Q:32 * Q + 3, :])

            prev = None
            late = {}
            for s in range(ns):
                if prev is not None:
                    stage_norm(prev)
                cur = stage_a(s)
                if s == 0:
                    # back-half weights: loaded while sample 0's front half runs
                    late["wb2"] = wload("wb2", [128, 2, 9, 128], F16, d_wb2, split=4)
                    late["bb2"] = wload("bb2", [128, 1], F32, d_bb2)
                    late["wdt"] = wload("wdt", [128, 16, 64], F16, d_wdt, split=2)
                    late["bdt"] = wload("bdt", [64, 1], F32, d_bdt)
                    late["wdc"] = wload("wdc", [64, 9, 3], F16, d_wdc)
                    late["bdc"] = wload("bdc", [128, 1], F32, d_bdc)
                if prev is not None:
                    stage_b(prev)
                stage_a2(cur)
                if s == ns - 1:
                    emit_rep(cur)
                prev = cur
            stage_norm(prev)
            stage_b(prev)

    nc.compile()
    return nc


def prep_static(ew1, eb1, ew2, eb2, bw1, bb1, qw, qb, kw, kb, vw, vb,
                gamma, bw2, bb2, dtw, dtb, dcw, dcb):
    """Host-side weight layout prep (shared across cores)."""
    f16 = np.float16
    f32 = np.float32
    out = {}
    wim = np.zeros((36, 64), np.float32)
    for dy in range(3):
        for dx in range(3):
            t = dy * 3 + dx
            wim[t * 4:t * 4 + 4, :] = ew1[:, :, dy, dx].T
    out["wim"] = wim.astype(f16)
    out["be1"] = eb1.reshape(64, 1).astype(f32)
    we2 = np.transpose(ew2, (1, 2, 3, 0)).reshape(64, 9, 128)
    out["we2"] = np.ascontiguousarray(we2).astype(f16)
    out["be2"] = eb2.reshape(128, 1).astype(f32)
    wb1 = np.transpose(bw1, (1, 2, 3, 0)).reshape(128, 9, 2, 128)
    out["wb1"] = np.ascontiguousarray(wb1).astype(f16)
    out["bb1"] = bb1.reshape(2, 128).T.astype(f32).copy()
    wq = qw[:, :, 0, 0].T.reshape(2, 128, 32).transpose(1, 0, 2)
    out["wq"] = np.ascontiguousarray(wq).astype(f16)
    out["bq"] = qb.reshape(32, 1).astype(f32)
    wk = kw[:, :, 0, 0].T.reshape(2, 128, 32).transpose(1, 0, 2)
    out["wk"] = np.ascontiguousarray(wk).astype(f16)
    out["bk"] = kb.reshape(32, 1).astype(f32)
    wv = vw[:, :, 0, 0].T.reshape(2, 128, 256).transpose(1, 0, 2)
    out["wv"] = np.ascontiguousarray(wv).astype(f16)
    g = float(np.asarray(gamma).reshape(-1)[0])
    out["gvb"] = (g * vb).reshape(2, 128).T.astype(f32).copy()
    out["gam"] = np.full((1, 1), g, f32)
    wb2_ = np.transpose(bw2, (1, 2, 3, 0)).reshape(2, 128, 9, 128).transpose(1, 0, 2, 3)
    out["wb2"] = np.ascontiguousarray(wb2_).astype(f16)
    out["bb2"] = bb2.reshape(128, 1).astype(f32)
    kmap = {(0, 0): 1, (0, 1): 3, (1, 0): 0, (1, 1): 2}
    wdt = np.zeros((128, 16, 64), np.float32)
    for py in range(2):
        for px in range(2):
            for dy2 in range(2):
                for dx2 in range(2):
                    ky = kmap[(py, dy2)]
                    kx = kmap[(px, dx2)]
                    wdt[:, (py * 2 + px) * 4 + dy2 * 2 + dx2, :] = dtw[:, :, ky, kx]
    out["wdt"] = wdt.astype(f16)
    out["bdt"] = dtb.reshape(64, 1).astype(f32)
    wdc = np.transpose(dcw, (1, 2, 3, 0)).reshape(64, 9, 3)
    out["wdc"] = np.ascontiguousarray(wdc).astype(f16)
    bdc = np.zeros((128, 1), f32)
    for Q in range(4):
        bdc[32 * Q:32 * Q + 3, 0] = dcb
    out["bdc"] = bdc
    return out


def pos_encoding():
    c = np.arange(2, dtype=np.float32)
    yy = np.arange(64, dtype=np.float32)
    ang = yy[None, :] / (10000.0 ** (2.0 * c / 4.0)).astype(np.float32)[:, None]
    pe = np.zeros((4, 64), np.float32)
    pe[0::2] = np.sin(ang)
    pe[1::2] = np.cos(ang)
    return pe


def build_m0(x_shard, le_shard):
    """x_shard [ns,3,64,64] f32, le_shard [ns,64,64] f32 -> [ns,36,64,64] f16."""
    ns = x_shard.shape[0]
    pe = pos_encoding()
    h0 = np.zeros((ns, 4, 66, 66), np.float32)
    h0[:, :3, 1:65, 1:65] = x_shard
    h0[:, 3, 1:65, 1:65] = le_shard
    h0[:, :, 1:65, 1:65] += pe[None, :, :, None]
    m0 = np.zeros((ns, 36, 64, 64), np.float32)
    for dy in range(3):
        for dx in range(3):
            t = dy * 3 + dx
            m0[:, t * 4:t * 4 + 4] = h0[:, :, dy:dy + 64, dx:dx + 64]
    # permute columns so e1's relu write is contiguous in the h1p plane layout:
    # first 32 cols -> odd x (plane0 slots xx1..32), last 32 -> even x (plane1 xx0..31)
    m0p = np.empty_like(m0)
    m0p[:, :, :, 0:32] = m0[:, :, :, 1::2]
    m0p[:, :, :, 32:64] = m0[:, :, :, 0::2]
    return m0p.astype(np.float16)


def make_in_maps(x, labels, label_emb, static):
    le = label_emb[labels].reshape(-1, 64, 64)
    in_maps = []
    for c in range(NCORES):
        sl = slice(c * NS, (c + 1) * NS)
        m = dict(static)
        m["m0"] = build_m0(x[sl], le[sl])
        in_maps.append(m)
    return in_maps


def kernel(x, t, labels, label_emb, ew1, eb1, ew2, eb2, bw1, bb1,
           qw, qb, kw, kb, vw, vb, gamma, bw2, bb2, dtw, dtb, dcw, dcb):
    del t
    x = np.asarray(x, np.float32)
    labels = np.asarray(labels)
    label_emb = np.asarray(label_emb, np.float32)
    static = prep_static(np.asarray(ew1), np.asarray(eb1), np.asarray(ew2),
                         np.asarray(eb2), np.asarray(bw1), np.asarray(bb1),
                         np.asarray(qw), np.asarray(qb), np.asarray(kw),
                         np.asarray(kb), np.asarray(vw), np.asarray(vb),
                         np.asarray(gamma), np.asarray(bw2), np.asarray(bb2),
                         np.asarray(dtw), np.asarray(dtb), np.asarray(dcw),
                         np.asarray(dcb))
    in_maps = make_in_maps(x, labels, label_emb, static)
    if "nc" not in _cache:
        _cache["nc"] = build_nc()
    nc = _cache["nc"]
    res = run_bass_kernel_spmd(nc, in_maps, core_ids=list(range(NCORES)))
    return np.concatenate([res.results[c]["out"] for c in range(NCORES)], axis=0)
